# revision 21
# baseline (speedup 1.0000x reference)
"""Cross-attention kernel for 8 Trainium2 NeuronCores (Bass/Tile, SPMD).

Problem: nn_CrossAttention (B=4, NQ=1024, NK=2048, C=1024, H=16, D=64), fp32.

Sharding: (batch x head-group) across the 8 cores. Core c handles batch
b = c//2 and heads h0 = (c%2)*8 .. h0+8 (column-parallel q/k/v projections,
row-parallel output projection). Each core emits a partial output
projection [NQ, C]; the host sums the two partials per batch (+ biases).

Device dataflow is fully "feature-major" (transposed): the host passes
x.T / y.T / W.T so every matmul contraction runs over the SBUF partition
axis with no on-device transposes:

  qT[o,t]  = sum_c wqT[c,o] * xT[c,t]          (o-blocks of 128 = 2 heads)
  kT[o,s]  = sum_c wkT[c,o] * yT[c,s]
  v[s,o]   = sum_c yT[c,s] * wvT[c,o]          (token-major, + ones column)
  ST[s,t]  = sum_d kT_h[d,s-chunk] * qT_h[d,t]   per head (K=64)
  P[s,t]   = exp(ST) * mask01[s,t]               (ACT exp from PSUM, DVE mul)
  outT_aug = sum_s v_aug_h[s,(d|1)] * P[s,t]     -> row 64 = sum_s P = denom
  outF     = outT_aug[0:64] / denom              (softmax normalization)
  partial[t,co] = sum_o outF[o,t-block] * wpT[o,co]

Schedule: fully software-pipelined. The attention loop processes the 8
heads sequentially; v-projection and the kT projections for later head
pairs are emitted as PE "filler" units inside the attention chunk loop so
the tensor engine stays busy while the ACT engine drains the exp stream
(exp of [128,1024] is ~1.1us vs ~0.85us of matmul per chunk, so attention
alone is ACT-bound). PSUM: 2 banks filler + 4 banks scores (2 in flight)
+ 2 banks attn-out accumulator = 8.

Bias handling (exact): bq is added on-device during the qT eviction;
bk shifts every score of a row equally -> softmax-invariant -> dropped;
bv passes through the softmax average exactly -> host adds bv @ Wp.T;
bp is added on the host.
"""

import os
import sys

if "/opt/trn_rl_repo" not in sys.path:
    sys.path.insert(0, "/opt/trn_rl_repo")

import numpy as np
import ml_dtypes

B, NQ, NK, C, H = 4, 1024, 2048, 1024, 16
D = C // H          # 64
HC = H // 2         # 8 heads per core
CO = HC * D         # 512 output dims per core
N_CORES = 8

_CACHE = {}


def _install_ntff_hook():
    """Register the axon NTFF profile hook (missing antenv.axon_hooks shim).
    Only needed when tracing; harmless otherwise."""
    import types

    if "antenv.axon_hooks" in sys.modules:
        return
    state = {"hook": None}
    mod = types.ModuleType("antenv.axon_hooks")
    mod.set_axon_ntff_profile_hook = lambda h: state.__setitem__("hook", h)
    mod.get_axon_ntff_profile_hook = lambda: state["hook"]
    sys.modules["antenv.axon_hooks"] = mod
    try:
        from trn_agent_boot.trn_boot import _ntff_profile_via_ctypes

        mod.set_axon_ntff_profile_hook(
            _ntff_profile_via_ctypes("/opt/axon/libaxon_pjrt.so")
        )
    except Exception:
        pass


def _build():
    import concourse.mybir as mybir
    import concourse.tile as tile
    from concourse import bacc

    F32 = mybir.dt.float32
    BF16 = mybir.dt.bfloat16
    Exp = mybir.ActivationFunctionType.Exp
    Copy = mybir.ActivationFunctionType.Copy

    nc = bacc.Bacc("TRN2", target_bir_lowering=False, debug=False,
                   num_devices=N_CORES)

    def din(name, shape, dt=BF16):
        return nc.dram_tensor(name, shape, dt, kind="ExternalInput").ap()

    xT = din("xT", [C, NQ])            # x[b].T
    yT = din("yT", [C, NK])            # y[b].T
    m01T = din("m01T", [NK, NQ], BF16)  # keep=1 / masked=0, transposed
    wqT = din("wqT", [C, CO])          # (Wq[rows]*scale).T
    wkT = din("wkT", [C, CO])
    wvT = din("wvT", [C, CO])
    wpT = din("wpT", [CO, C])          # Wp[:, rows].T
    bqv = din("bq", [CO], mybir.dt.float32)   # scaled bq slice
    part = nc.dram_tensor("part", [NQ, C], BF16, kind="ExternalOutput").ap()

    LA = 2  # score->attn-v lookahead (chunks in flight)

    with tile.TileContext(nc) as tc:
        with (
            tc.tile_pool(name="persist", bufs=1) as persist,
            tc.tile_pool(name="work_e", bufs=4) as pe_,
            tc.tile_pool(name="work_p", bufs=4) as pp_,
            tc.tile_pool(name="work_d", bufs=1) as pd_,
            tc.tile_pool(name="ps_proj", bufs=2, space="PSUM") as ps_proj,
            tc.tile_pool(name="ps_st", bufs=2, space="PSUM") as ps_st,
            tc.tile_pool(name="ps_out", bufs=1, space="PSUM") as ps_out,
        ):
            # ---- persistent tiles --------------------------------------
            kT_sb = [persist.tile([128, NK], BF16, tag=f"kT{i}", name=f"kT{i}") for i in range(4)]
            v_sb = [persist.tile([128, HC, D + 1], BF16, tag=f"v{i}", name=f"v{i}") for i in range(16)]
            qT_sb = [persist.tile([128, NQ], BF16, tag=f"qT{i}", name=f"qT{i}") for i in range(4)]
            outF_sb = [persist.tile([128, NQ], BF16, tag=f"oF{i}", name=f"oF{i}") for i in range(4)]
            wp_sb = [persist.tile([128, C], BF16, tag=f"wp{i}", name=f"wp{i}") for i in range(4)]
            bq_sb = [persist.tile([128, 1], F32, tag=f"bq{i}", name=f"bq{i}") for i in range(4)]
            x_sb = [persist.tile([128, NQ], BF16, tag=f"x{c}", name=f"x{c}") for c in range(8)]
            wq_sb = [persist.tile([128, CO], BF16, tag=f"wq{c}", name=f"wq{c}") for c in range(8)]
            y_sb = [persist.tile([128, NK], BF16, tag=f"y{c}", name=f"y{c}") for c in range(8)]
            wk_sb = [persist.tile([128, CO], BF16, tag=f"wk{c}", name=f"wk{c}") for c in range(8)]
            wv_sb = [persist.tile([128, CO], BF16, tag=f"wv{c}", name=f"wv{c}") for c in range(8)]
            m_sb = [persist.tile([128, NQ], BF16, tag=f"m{i}", name=f"m{i}") for i in range(16)]

            # ---- all input DMAs, in first-needed order -----------------
            # kT(ob0) is the first PE consumer -> y/wk first, then x/wq for
            # the q projection, then wv + mask chunks for the first pass.
            # Spread across the idle engine queues so descriptor generation
            # doesn't serialize the lead-in.
            for cc in range(8):
                nc.sync.dma_start(wk_sb[cc][:], wkT[cc * 128:(cc + 1) * 128, :])
            for q4 in range(4):          # quarter-major: kT(0,q4) starts as
                for cc in range(8):      # soon as its y columns land
                    nc.sync.dma_start(
                        y_sb[cc][:, q4 * 512:(q4 + 1) * 512],
                        yT[cc * 128:(cc + 1) * 128, q4 * 512:(q4 + 1) * 512])
            for ob in range(4):
                nc.sync.dma_start(bq_sb[ob][:], bqv[ob * 128:(ob + 1) * 128][:, None])
            for cc in range(8):
                nc.sync.dma_start(wq_sb[cc][:], wqT[cc * 128:(cc + 1) * 128, :])
            nc.sync.dma_start(m_sb[0][:], m01T[0:128, :])
            for h2 in range(2):
                for cc in range(8):
                    nc.sync.dma_start(
                        x_sb[cc][:, h2 * 512:(h2 + 1) * 512],
                        xT[cc * 128:(cc + 1) * 128, h2 * 512:(h2 + 1) * 512])
            for cc in range(8):
                nc.sync.dma_start(wv_sb[cc][:], wvT[cc * 128:(cc + 1) * 128, :])
            for sc in range(1, 16):
                nc.sync.dma_start(m_sb[sc][:], m01T[sc * 128:(sc + 1) * 128, :])
            for ob in range(4):
                nc.sync.dma_start(wp_sb[ob][:], wpT[ob * 128:(ob + 1) * 128, :])

            # ones column of v_aug (last index -> denominator lands on PSUM
            # partition 64, aligned, readable by the reciprocal directly);
            # written once, v evictions leave it alone
            for sc in range(16):
                nc.vector.memset(v_sb[sc][:, :, D:D + 1], 1.0)

            # ---- PE unit emitters --------------------------------------
            # each projection unit is split into two "chunks" of 4 matmuls
            # so the filler can be paced finely inside the attention loop
            def _proj_chunks(make_stationary, moving, evict, name):
                box = {}

                def c1():
                    box["t"] = ps_proj.tile([128, 512], F32, tag="pps",
                                            name=name)
                    for cc in range(4):
                        nc.tensor.matmul(
                            box["t"][:], make_stationary(cc), moving(cc),
                            start=(cc == 0), stop=False,
                            skip_group_check=True,
                        )

                def c2():
                    for cc in range(4, 8):
                        nc.tensor.matmul(
                            box["t"][:], make_stationary(cc), moving(cc),
                            start=False, stop=(cc == 7),
                            skip_group_check=True,
                        )
                    evict(box["t"])

                return [c1, c2]

            def qproj_chunks(ob, tc2):
                return _proj_chunks(
                    lambda cc: wq_sb[cc][:, ob * 128:(ob + 1) * 128],
                    lambda cc: x_sb[cc][:, tc2 * 512:(tc2 + 1) * 512],
                    lambda t: nc.vector.tensor_scalar_add(
                        qT_sb[ob][:, tc2 * 512:(tc2 + 1) * 512],
                        t[:], bq_sb[ob][:]),
                    f"qps{ob}_{tc2}",
                )

            def kT_chunks(ob, sc4):
                return _proj_chunks(
                    lambda cc: wk_sb[cc][:, ob * 128:(ob + 1) * 128],
                    lambda cc: y_sb[cc][:, sc4 * 512:(sc4 + 1) * 512],
                    lambda t: nc.vector.tensor_copy(
                        kT_sb[ob][:, sc4 * 512:(sc4 + 1) * 512], t[:]),
                    f"kps{ob}_{sc4}",
                )

            def v_chunks(sc):
                return _proj_chunks(
                    lambda cc: y_sb[cc][:, sc * 128:(sc + 1) * 128],
                    lambda cc: wv_sb[cc][:],
                    lambda t: nc.vector.tensor_copy(v_sb[sc][:, :, 0:D],
                                                    t[:]),
                    f"vps{sc}",
                )

            def dummy_unit(tag):
                # keeps the PE issue queue deep in filler-less passes so the
                # tensor engine holds its high p-state; result never read
                dps = ps_proj.tile([128, 512], F32, tag="pps", name=tag)
                nc.tensor.matmul(
                    dps[:], kT_sb[0][0:64, 0:128], qT_sb[0][0:64, 0:512],
                    start=True, stop=True, skip_group_check=True,
                )

            # ---- startup: kT(ob0) + qT(ob0) only -----------------------
            for sc4 in range(4):
                for c in kT_chunks(0, sc4):
                    c()
            for tc2 in range(2):
                for c in qproj_chunks(0, tc2):
                    c()

            # filler schedule: pass index (hp, h2) -> list of chunk thunks.
            # v units must all land in pass (0,0) (attn-v of hp0/h0 consumes
            # v chunk sc at iteration sc+LA, and chunk pacing keeps unit sc
            # no later than iteration sc); kT(ob)/qT(ob) must finish before
            # pass (ob, 0) begins.  Spread as evenly as dependencies allow
            # so every pass keeps the PE issue queue deep (p-state).
            filler = {
                (0, 0): [c for sc in range(16) for c in v_chunks(sc)],
                (0, 1): [c for u in (qproj_chunks(1, 0), kT_chunks(1, 0),
                                     qproj_chunks(1, 1), kT_chunks(1, 1),
                                     kT_chunks(1, 2), kT_chunks(1, 3))
                         for c in u],
                (1, 0): [c for u in (qproj_chunks(2, 0), kT_chunks(2, 0),
                                     kT_chunks(2, 1)) for c in u],
                (1, 1): [c for u in (qproj_chunks(2, 1), kT_chunks(2, 2),
                                     kT_chunks(2, 3)) for c in u],
                (2, 0): [c for u in (qproj_chunks(3, 0), kT_chunks(3, 0),
                                     kT_chunks(3, 1)) for c in u],
                (2, 1): [c for u in (qproj_chunks(3, 1), kT_chunks(3, 2),
                                     kT_chunks(3, 3)) for c in u],
            }

            # ---- attention: heads sequential, pipelined chunks ---------
            for hp in range(4):
                for h2 in range(2):
                    p0 = h2 * 64
                    chunks = filler.get((hp, h2), [])
                    n_chunks = len(chunks)
                    popped = 0
                    outps = ps_out.tile([D + 1, NQ], F32, tag="outps",
                                        name=f"outps{hp}_{h2}")
                    pts = {}
                    for it in range(16 + LA):
                        if it >= LA:
                            # attn-v first: its inputs (pt, v) are LA chunks
                            # old, so the PE never stalls entering the iter
                            sc = it - LA
                            pt = pts.pop(sc)
                            for tc2 in range(2):
                                nc.tensor.matmul(
                                    outps[:, tc2 * 512:(tc2 + 1) * 512],
                                    v_sb[sc][:, 2 * hp + h2, :],
                                    pt[:, tc2 * 512:(tc2 + 1) * 512],
                                    start=(sc == 0), stop=(sc == 15),
                                    skip_group_check=True,
                                )
                        if it < 16:
                            sc = it
                            # PE filler ahead of this chunk's score matmuls,
                            # paced evenly across the 16 iterations
                            want = -(-n_chunks * (it + 1) // 16)  # ceil
                            while popped < want:
                                chunks[popped]()
                                popped += 1
                            if n_chunks == 0:
                                dummy_unit(f"dmy{hp}_{h2}_{it}")
                            stp = ps_st.tile([128, NQ], F32, tag="stp",
                                             name=f"stp{hp}_{h2}_{sc}")
                            for tc2 in range(2):
                                nc.tensor.matmul(
                                    stp[:, tc2 * 512:(tc2 + 1) * 512],
                                    kT_sb[hp][p0:p0 + 64,
                                              sc * 128:(sc + 1) * 128],
                                    qT_sb[hp][p0:p0 + 64,
                                              tc2 * 512:(tc2 + 1) * 512],
                                    start=True, stop=True,
                                    skip_group_check=True,
                                )
                            e = pe_.tile([128, NQ], BF16, tag="e")
                            nc.scalar.activation(e[:], stp[:], Exp)
                            pt = pp_.tile([128, NQ], BF16, tag="pt")
                            nc.vector.tensor_mul(pt[:], e[:], m_sb[sc][:])
                            pts[sc] = pt
                    # normalization: outF = outT[0:64] / outT[64] (denom
                    # row).  The denom row is copied off PSUM first (regular
                    # DVE copy, cross-partition-base to land on partition 0
                    # for the custom-DVE reciprocal), so the reciprocal +
                    # gpsimd broadcast overlap the raw eviction.
                    r0 = pd_.tile([1, NQ], F32, tag="r0")
                    nc.vector.tensor_copy(r0[:], outps[D:D + 1, :])
                    rc = pd_.tile([1, NQ], F32, tag="rc")
                    nc.vector.reciprocal_approx_fast(rc[:], r0[:])
                    raw = pd_.tile([D + 1, NQ], F32, tag="raw")
                    nc.vector.tensor_copy(raw[:], outps[:])
                    rin = pd_.tile([64, NQ], F32, tag="rin")
                    nc.gpsimd.partition_broadcast(rin[:], rc[:])
                    nc.vector.tensor_mul(
                        outF_sb[hp][p0:p0 + 64, :], raw[0:D, :], rin[:])

            # ---- output projection -------------------------------------
            # tb0..tb2 pre-accumulate oc=0..2 while the final normalize
            # (which produces outF[3] rows 64:128) is still in flight, so
            # the PE keeps running through the attention->projection seam.
            with tc.tile_pool(name="proj", bufs=2) as prj:
                def d_mms(tb, pps, oc_lo, oc_hi):
                    for oc in range(oc_lo, oc_hi):
                        for co in range(2):
                            nc.tensor.matmul(
                                pps[:, co * 512:(co + 1) * 512],
                                outF_sb[oc][:, tb * 128:(tb + 1) * 128],
                                wp_sb[oc][:, co * 512:(co + 1) * 512],
                                start=(oc == 0), stop=(oc == 3),
                                skip_group_check=True,
                            )

                pps_of = {}
                for tb in range(2):
                    pps_of[tb] = ps_st.tile([128, NQ], F32, tag="stp",
                                            name=f"pps{tb}")
                    d_mms(tb, pps_of[tb], 0, 3)
                pair2 = [ps_proj.tile([128, 512], F32, tag="pps",
                                      name=f"ppre{co}") for co in range(2)]
                for oc in range(3):
                    for co in range(2):
                        nc.tensor.matmul(
                            pair2[co][:],
                            outF_sb[oc][:, 2 * 128:3 * 128],
                            wp_sb[oc][:, co * 512:(co + 1) * 512],
                            start=(oc == 0), stop=False,
                            skip_group_check=True,
                        )
                for tb in range(8):
                    po = prj.tile([128, C], BF16, tag="po")
                    if tb in pps_of:
                        pps = pps_of[tb]
                        d_mms(tb, pps, 3, 4)
                        nc.vector.tensor_copy(po[:], pps[:])
                    elif tb == 2:
                        for co in range(2):
                            nc.tensor.matmul(
                                pair2[co][:],
                                outF_sb[3][:, 2 * 128:3 * 128],
                                wp_sb[3][:, co * 512:(co + 1) * 512],
                                start=False, stop=True,
                                skip_group_check=True,
                            )
                            nc.vector.tensor_copy(
                                po[:, co * 512:(co + 1) * 512], pair2[co][:])
                    else:
                        pps = ps_st.tile([128, NQ], F32, tag="stp",
                                         name=f"pps{tb}")
                        d_mms(tb, pps, 0, 4)
                        nc.vector.tensor_copy(po[:], pps[:])
                    nc.sync.dma_start(part[tb * 128:(tb + 1) * 128, :], po[:])

    nc.compile()
    return nc


def _get_nc():
    if "nc" not in _CACHE:
        _CACHE["nc"] = _build()
    return _CACHE["nc"]


def kernel(x, y, mask, Wq, bq, Wkv, bkv, Wp, bp):
    _install_ntff_hook()
    from concourse.bass_utils import run_bass_kernel_spmd

    x = np.asarray(x, dtype=np.float32)
    y = np.asarray(y, dtype=np.float32)
    mask = np.asarray(mask)
    Wq = np.asarray(Wq, dtype=np.float32)
    Wkv = np.asarray(Wkv, dtype=np.float32)
    Wp = np.asarray(Wp, dtype=np.float32)
    bq = np.asarray(bq, dtype=np.float32)
    bkv = np.asarray(bkv, dtype=np.float32)
    bp = np.asarray(bp, dtype=np.float32)

    scale = D ** -0.5
    bf16 = ml_dtypes.bfloat16
    xTs = [np.ascontiguousarray(x[b].T).astype(bf16) for b in range(B)]
    yTs = [np.ascontiguousarray(y[b].T).astype(bf16) for b in range(B)]
    m01Ts = [
        np.ascontiguousarray((~mask[b, 0]).T.astype(np.float32)).astype(
            ml_dtypes.bfloat16)
        for b in range(B)
    ]
    wqTs, wkTs, wvTs, wpTs, bqs = [], [], [], [], []
    for hg in range(2):
        rows = slice(hg * CO, hg * CO + CO)
        wqTs.append(np.ascontiguousarray((Wq[rows] * scale).T).astype(bf16))
        wkTs.append(np.ascontiguousarray(Wkv[rows].T).astype(bf16))
        wvTs.append(np.ascontiguousarray(Wkv[C + hg * CO: C + hg * CO + CO].T).astype(bf16))
        wpTs.append(np.ascontiguousarray(Wp[:, rows].T).astype(bf16))
        bqs.append(np.ascontiguousarray(bq[rows] * scale))

    in_maps = []
    for c in range(N_CORES):
        b, hg = divmod(c, 2)
        in_maps.append({
            "xT": xTs[b], "yT": yTs[b], "m01T": m01Ts[b],
            "wqT": wqTs[hg], "wkT": wkTs[hg], "wvT": wvTs[hg],
            "wpT": wpTs[hg], "bq": bqs[hg],
        })

    nc = _get_nc()
    trace = os.environ.get("CC_ATTN_TRACE", "") == "1"
    res = run_bass_kernel_spmd(nc, in_maps, core_ids=list(range(N_CORES)),
                               trace=trace)
    _CACHE["last_result"] = res

    # host gather: sum the two head-group partials per batch + exact bias folds
    bias = bkv[C:] @ Wp.T + bp  # v-bias passes through softmax exactly
    out = np.empty((B, NQ, C), dtype=np.float32)
    for b in range(B):
        out[b] = (res.results[2 * b]["part"].astype(np.float32)
                  + res.results[2 * b + 1]["part"].astype(np.float32) + bias)
    return out


# revision 22
# speedup vs baseline: 1.0689x; 1.0689x over previous
"""Cross-attention kernel for 8 Trainium2 NeuronCores (Bass/Tile, SPMD).

Problem: nn_CrossAttention (B=4, NQ=1024, NK=2048, C=1024, H=16, D=64), fp32.

Sharding: (batch x head-group) across the 8 cores. Core c handles batch
b = c//2 and heads h0 = (c%2)*8 .. h0+8 (column-parallel q/k/v projections,
row-parallel output projection). Each core emits a partial output
projection [NQ, C]; the host sums the two partials per batch (+ biases).

Device dataflow is fully "feature-major" (transposed): the host passes
x.T / y.T / W.T so every matmul contraction runs over the SBUF partition
axis with no on-device transposes:

  qT[o,t]  = sum_c wqT[c,o] * xT[c,t]          (o-blocks of 128 = 2 heads)
  kT[o,s]  = sum_c wkT[c,o] * yT[c,s]
  v[s,o]   = sum_c yT[c,s] * wvT[c,o]          (token-major, + ones column)
  ST[s,t]  = sum_d kT_h[d,s-chunk] * qT_h[d,t]   per head (K=64)
  P[s,t]   = exp(ST) * mask01[s,t]               (ACT exp from PSUM, DVE mul)
  outT_aug = sum_s v_aug_h[s,(d|1)] * P[s,t]     -> row 64 = sum_s P = denom
  outF     = outT_aug[0:64] / denom              (softmax normalization)
  partial[t,co] = sum_o outF[o,t-block] * wpT[o,co]

Schedule: fully software-pipelined. The attention loop processes the 8
heads sequentially; v-projection and the kT projections for later head
pairs are emitted as PE "filler" units inside the attention chunk loop so
the tensor engine stays busy while the ACT engine drains the exp stream
(exp of [128,1024] is ~1.1us vs ~0.85us of matmul per chunk, so attention
alone is ACT-bound). PSUM: 2 banks filler + 4 banks scores (2 in flight)
+ 2 banks attn-out accumulator = 8.

Bias handling (exact): bq is added on-device during the qT eviction;
bk shifts every score of a row equally -> softmax-invariant -> dropped;
bv passes through the softmax average exactly -> host adds bv @ Wp.T;
bp is added on the host.
"""

import os
import sys

if "/opt/trn_rl_repo" not in sys.path:
    sys.path.insert(0, "/opt/trn_rl_repo")

import numpy as np
import ml_dtypes

B, NQ, NK, C, H = 4, 1024, 2048, 1024, 16
D = C // H          # 64
HC = H // 2         # 8 heads per core
CO = HC * D         # 512 output dims per core
N_CORES = 8

_CACHE = {}


def _install_ntff_hook():
    """Register the axon NTFF profile hook (missing antenv.axon_hooks shim).
    Only needed when tracing; harmless otherwise."""
    import types

    if "antenv.axon_hooks" in sys.modules:
        return
    state = {"hook": None}
    mod = types.ModuleType("antenv.axon_hooks")
    mod.set_axon_ntff_profile_hook = lambda h: state.__setitem__("hook", h)
    mod.get_axon_ntff_profile_hook = lambda: state["hook"]
    sys.modules["antenv.axon_hooks"] = mod
    try:
        from trn_agent_boot.trn_boot import _ntff_profile_via_ctypes

        mod.set_axon_ntff_profile_hook(
            _ntff_profile_via_ctypes("/opt/axon/libaxon_pjrt.so")
        )
    except Exception:
        pass


def _build():
    import concourse.mybir as mybir
    import concourse.tile as tile
    from concourse import bacc

    F32 = mybir.dt.float32
    BF16 = mybir.dt.bfloat16
    Exp = mybir.ActivationFunctionType.Exp
    Copy = mybir.ActivationFunctionType.Copy

    nc = bacc.Bacc("TRN2", target_bir_lowering=False, debug=False,
                   num_devices=N_CORES)

    def din(name, shape, dt=BF16):
        return nc.dram_tensor(name, shape, dt, kind="ExternalInput").ap()

    xT = din("xT", [C, NQ])            # x[b].T
    yT = din("yT", [C, NK])            # y[b].T
    m01T = din("m01T", [NK, NQ], BF16)  # keep=1 / masked=0, transposed
    wqT = din("wqT", [C, CO])          # (Wq[rows]*scale).T
    wkT = din("wkT", [C, CO])
    wvT = din("wvT", [C, CO])
    wpT = din("wpT", [CO, C])          # Wp[:, rows].T
    bqv = din("bq", [CO], mybir.dt.float32)   # scaled bq slice
    part = nc.dram_tensor("part", [NQ, C], BF16, kind="ExternalOutput").ap()

    LA = 2  # score->attn-v lookahead (chunks in flight)

    with tile.TileContext(nc) as tc:
        with (
            tc.tile_pool(name="persist", bufs=1) as persist,
            tc.tile_pool(name="work_e", bufs=4) as pe_,
            tc.tile_pool(name="work_p", bufs=4) as pp_,
            tc.tile_pool(name="work_d", bufs=1) as pd_,
            tc.tile_pool(name="ps_proj", bufs=2, space="PSUM") as ps_proj,
            tc.tile_pool(name="ps_st", bufs=2, space="PSUM") as ps_st,
            tc.tile_pool(name="ps_out", bufs=1, space="PSUM") as ps_out,
        ):
            # ---- persistent tiles --------------------------------------
            kT_sb = [persist.tile([128, NK], BF16, tag=f"kT{i}", name=f"kT{i}") for i in range(4)]
            v_sb = [persist.tile([128, HC, D + 1], BF16, tag=f"v{i}", name=f"v{i}") for i in range(16)]
            qT_sb = [persist.tile([128, NQ], BF16, tag=f"qT{i}", name=f"qT{i}") for i in range(4)]
            outF_sb = [persist.tile([128, NQ], BF16, tag=f"oF{i}", name=f"oF{i}") for i in range(4)]
            wp_sb = [persist.tile([128, C], BF16, tag=f"wp{i}", name=f"wp{i}") for i in range(4)]
            bq_sb = [persist.tile([128, 1], F32, tag=f"bq{i}", name=f"bq{i}") for i in range(4)]
            x_sb = [persist.tile([128, NQ], BF16, tag=f"x{c}", name=f"x{c}") for c in range(8)]
            wq_sb = [persist.tile([128, CO], BF16, tag=f"wq{c}", name=f"wq{c}") for c in range(8)]
            y_sb = [persist.tile([128, NK], BF16, tag=f"y{c}", name=f"y{c}") for c in range(8)]
            wk_sb = [persist.tile([128, CO], BF16, tag=f"wk{c}", name=f"wk{c}") for c in range(8)]
            wv_sb = [persist.tile([128, CO], BF16, tag=f"wv{c}", name=f"wv{c}") for c in range(8)]
            m_sb = [persist.tile([128, NQ], BF16, tag=f"m{i}", name=f"m{i}") for i in range(16)]

            # ---- all input DMAs, in first-needed order -----------------
            # kT(ob0) is the first PE consumer -> y/wk first, then x/wq for
            # the q projection, then wv + mask chunks for the first pass.
            # Spread across the idle engine queues so descriptor generation
            # doesn't serialize the lead-in.
            for cc in range(8):
                nc.sync.dma_start(y_sb[cc][:], yT[cc * 128:(cc + 1) * 128, :])
                nc.sync.dma_start(wk_sb[cc][:], wkT[cc * 128:(cc + 1) * 128, :])
            for ob in range(4):
                nc.sync.dma_start(bq_sb[ob][:], bqv[ob * 128:(ob + 1) * 128][:, None])
            nc.sync.dma_start(m_sb[0][:], m01T[0:128, :])
            for cc in range(8):
                nc.sync.dma_start(x_sb[cc][:], xT[cc * 128:(cc + 1) * 128, :])
                nc.sync.dma_start(wq_sb[cc][:], wqT[cc * 128:(cc + 1) * 128, :])
            for cc in range(8):
                nc.sync.dma_start(wv_sb[cc][:], wvT[cc * 128:(cc + 1) * 128, :])
            for sc in range(1, 16):
                nc.sync.dma_start(m_sb[sc][:], m01T[sc * 128:(sc + 1) * 128, :])
            for ob in range(4):
                nc.sync.dma_start(wp_sb[ob][:], wpT[ob * 128:(ob + 1) * 128, :])

            # ones column of v_aug (last index -> denominator lands on PSUM
            # partition 64, aligned, readable by the reciprocal directly);
            # written once, v evictions leave it alone
            for sc in range(16):
                nc.vector.memset(v_sb[sc][:, :, D:D + 1], 1.0)

            # ---- PE unit emitters --------------------------------------
            # each projection unit is split into two "chunks" of 4 matmuls
            # so the filler can be paced finely inside the attention loop
            def _proj_chunks(make_stationary, moving, evict, name):
                box = {}

                def c1():
                    box["t"] = ps_proj.tile([128, 512], F32, tag="pps",
                                            name=name)
                    for cc in range(4):
                        nc.tensor.matmul(
                            box["t"][:], make_stationary(cc), moving(cc),
                            start=(cc == 0), stop=False,
                            skip_group_check=True,
                        )

                def c2():
                    for cc in range(4, 8):
                        nc.tensor.matmul(
                            box["t"][:], make_stationary(cc), moving(cc),
                            start=False, stop=(cc == 7),
                            skip_group_check=True,
                        )
                    evict(box["t"])

                return [c1, c2]

            def qproj_chunks(ob, tc2):
                return _proj_chunks(
                    lambda cc: wq_sb[cc][:, ob * 128:(ob + 1) * 128],
                    lambda cc: x_sb[cc][:, tc2 * 512:(tc2 + 1) * 512],
                    lambda t: nc.vector.tensor_scalar_add(
                        qT_sb[ob][:, tc2 * 512:(tc2 + 1) * 512],
                        t[:], bq_sb[ob][:]),
                    f"qps{ob}_{tc2}",
                )

            def kT_chunks(ob, sc4):
                return _proj_chunks(
                    lambda cc: wk_sb[cc][:, ob * 128:(ob + 1) * 128],
                    lambda cc: y_sb[cc][:, sc4 * 512:(sc4 + 1) * 512],
                    lambda t: nc.vector.tensor_copy(
                        kT_sb[ob][:, sc4 * 512:(sc4 + 1) * 512], t[:]),
                    f"kps{ob}_{sc4}",
                )

            def v_chunks(sc):
                return _proj_chunks(
                    lambda cc: y_sb[cc][:, sc * 128:(sc + 1) * 128],
                    lambda cc: wv_sb[cc][:],
                    lambda t: nc.vector.tensor_copy(v_sb[sc][:, :, 0:D],
                                                    t[:]),
                    f"vps{sc}",
                )

            def dummy_unit(tag):
                # keeps the PE issue queue deep in filler-less passes so the
                # tensor engine holds its high p-state; result never read
                dps = ps_proj.tile([128, 512], F32, tag="pps", name=tag)
                nc.tensor.matmul(
                    dps[:], kT_sb[0][0:64, 0:128], qT_sb[0][0:64, 0:512],
                    start=True, stop=True, skip_group_check=True,
                )

            # ---- startup: kT(ob0) + qT(ob0) only -----------------------
            for sc4 in range(4):
                for c in kT_chunks(0, sc4):
                    c()
            for tc2 in range(2):
                for c in qproj_chunks(0, tc2):
                    c()

            # filler schedule: pass index (hp, h2) -> list of chunk thunks.
            # v units must all land in pass (0,0) (attn-v of hp0/h0 consumes
            # v chunk sc at iteration sc+LA, and chunk pacing keeps unit sc
            # no later than iteration sc); kT(ob)/qT(ob) must finish before
            # pass (ob, 0) begins.  Spread as evenly as dependencies allow
            # so every pass keeps the PE issue queue deep (p-state).
            filler = {
                (0, 0): [c for sc in range(16) for c in v_chunks(sc)],
                (0, 1): [c for u in (qproj_chunks(1, 0), kT_chunks(1, 0),
                                     qproj_chunks(1, 1), kT_chunks(1, 1),
                                     kT_chunks(1, 2), kT_chunks(1, 3))
                         for c in u],
                (1, 0): [c for u in (qproj_chunks(2, 0), kT_chunks(2, 0),
                                     kT_chunks(2, 1)) for c in u],
                (1, 1): [c for u in (qproj_chunks(2, 1), kT_chunks(2, 2),
                                     kT_chunks(2, 3)) for c in u],
                (2, 0): [c for u in (qproj_chunks(3, 0), kT_chunks(3, 0),
                                     kT_chunks(3, 1)) for c in u],
                (2, 1): [c for u in (qproj_chunks(3, 1), kT_chunks(3, 2),
                                     kT_chunks(3, 3)) for c in u],
            }

            # ---- attention: heads sequential, pipelined chunks ---------
            for hp in range(4):
                for h2 in range(2):
                    p0 = h2 * 64
                    chunks = filler.get((hp, h2), [])
                    n_chunks = len(chunks)
                    popped = 0
                    outps = ps_out.tile([D + 1, NQ], F32, tag="outps",
                                        name=f"outps{hp}_{h2}")
                    pts = {}
                    for it in range(16 + LA):
                        if it >= LA:
                            # attn-v first: its inputs (pt, v) are LA chunks
                            # old, so the PE never stalls entering the iter
                            sc = it - LA
                            pt = pts.pop(sc)
                            for tc2 in range(2):
                                nc.tensor.matmul(
                                    outps[:, tc2 * 512:(tc2 + 1) * 512],
                                    v_sb[sc][:, 2 * hp + h2, :],
                                    pt[:, tc2 * 512:(tc2 + 1) * 512],
                                    start=(sc == 0), stop=(sc == 15),
                                    skip_group_check=True,
                                )
                        if it < 16:
                            sc = it
                            # PE filler ahead of this chunk's score matmuls,
                            # paced evenly across the 16 iterations
                            want = -(-n_chunks * (it + 1) // 16)  # ceil
                            while popped < want:
                                chunks[popped]()
                                popped += 1
                            if n_chunks == 0:
                                dummy_unit(f"dmy{hp}_{h2}_{it}")
                            stp = ps_st.tile([128, NQ], F32, tag="stp",
                                             name=f"stp{hp}_{h2}_{sc}")
                            for tc2 in range(2):
                                nc.tensor.matmul(
                                    stp[:, tc2 * 512:(tc2 + 1) * 512],
                                    kT_sb[hp][p0:p0 + 64,
                                              sc * 128:(sc + 1) * 128],
                                    qT_sb[hp][p0:p0 + 64,
                                              tc2 * 512:(tc2 + 1) * 512],
                                    start=True, stop=True,
                                    skip_group_check=True,
                                )
                            e = pe_.tile([128, NQ], BF16, tag="e")
                            nc.scalar.activation(e[:], stp[:], Exp)
                            pt = pp_.tile([128, NQ], BF16, tag="pt")
                            nc.vector.tensor_mul(pt[:], e[:], m_sb[sc][:])
                            pts[sc] = pt
                    # normalization: outF = outT[0:64] / outT[64] (denom
                    # row).  The denom row is copied off PSUM first (regular
                    # DVE copy, cross-partition-base to land on partition 0
                    # for the custom-DVE reciprocal), so the reciprocal +
                    # gpsimd broadcast overlap the raw eviction.
                    r0 = pd_.tile([1, NQ], F32, tag="r0")
                    nc.vector.tensor_copy(r0[:], outps[D:D + 1, :])
                    rc = pd_.tile([1, NQ], F32, tag="rc")
                    nc.vector.reciprocal_approx_fast(rc[:], r0[:])
                    raw = pd_.tile([D + 1, NQ], F32, tag="raw")
                    nc.vector.tensor_copy(raw[:], outps[:])
                    rin = pd_.tile([64, NQ], F32, tag="rin")
                    nc.gpsimd.partition_broadcast(rin[:], rc[:])
                    nc.vector.tensor_mul(
                        outF_sb[hp][p0:p0 + 64, :], raw[0:D, :], rin[:])

            # ---- output projection -------------------------------------
            # tb0..tb2 pre-accumulate oc=0..2 while the final normalize
            # (which produces outF[3] rows 64:128) is still in flight, so
            # the PE keeps running through the attention->projection seam.
            with tc.tile_pool(name="proj", bufs=2) as prj:
                def d_mms(tb, pps, oc_lo, oc_hi):
                    for oc in range(oc_lo, oc_hi):
                        for co in range(2):
                            nc.tensor.matmul(
                                pps[:, co * 512:(co + 1) * 512],
                                outF_sb[oc][:, tb * 128:(tb + 1) * 128],
                                wp_sb[oc][:, co * 512:(co + 1) * 512],
                                start=(oc == 0), stop=(oc == 3),
                                skip_group_check=True,
                            )

                pps_of = {}
                for tb in range(2):
                    pps_of[tb] = ps_st.tile([128, NQ], F32, tag="stp",
                                            name=f"pps{tb}")
                    d_mms(tb, pps_of[tb], 0, 3)
                pair2 = [ps_proj.tile([128, 512], F32, tag="pps",
                                      name=f"ppre{co}") for co in range(2)]
                for oc in range(3):
                    for co in range(2):
                        nc.tensor.matmul(
                            pair2[co][:],
                            outF_sb[oc][:, 2 * 128:3 * 128],
                            wp_sb[oc][:, co * 512:(co + 1) * 512],
                            start=(oc == 0), stop=False,
                            skip_group_check=True,
                        )
                for tb in range(8):
                    po = prj.tile([128, C], BF16, tag="po")
                    if tb in pps_of:
                        pps = pps_of[tb]
                        d_mms(tb, pps, 3, 4)
                        nc.vector.tensor_copy(po[:], pps[:])
                    elif tb == 2:
                        for co in range(2):
                            nc.tensor.matmul(
                                pair2[co][:],
                                outF_sb[3][:, 2 * 128:3 * 128],
                                wp_sb[3][:, co * 512:(co + 1) * 512],
                                start=False, stop=True,
                                skip_group_check=True,
                            )
                            nc.vector.tensor_copy(
                                po[:, co * 512:(co + 1) * 512], pair2[co][:])
                    else:
                        pps = ps_st.tile([128, NQ], F32, tag="stp",
                                         name=f"pps{tb}")
                        d_mms(tb, pps, 0, 4)
                        nc.vector.tensor_copy(po[:], pps[:])
                    nc.sync.dma_start(part[tb * 128:(tb + 1) * 128, :], po[:])

    nc.compile()
    return nc


def _get_nc():
    if "nc" not in _CACHE:
        _CACHE["nc"] = _build()
    return _CACHE["nc"]


def kernel(x, y, mask, Wq, bq, Wkv, bkv, Wp, bp):
    _install_ntff_hook()
    from concourse.bass_utils import run_bass_kernel_spmd

    x = np.asarray(x, dtype=np.float32)
    y = np.asarray(y, dtype=np.float32)
    mask = np.asarray(mask)
    Wq = np.asarray(Wq, dtype=np.float32)
    Wkv = np.asarray(Wkv, dtype=np.float32)
    Wp = np.asarray(Wp, dtype=np.float32)
    bq = np.asarray(bq, dtype=np.float32)
    bkv = np.asarray(bkv, dtype=np.float32)
    bp = np.asarray(bp, dtype=np.float32)

    scale = D ** -0.5
    bf16 = ml_dtypes.bfloat16
    xTs = [np.ascontiguousarray(x[b].T).astype(bf16) for b in range(B)]
    yTs = [np.ascontiguousarray(y[b].T).astype(bf16) for b in range(B)]
    m01Ts = [
        np.ascontiguousarray((~mask[b, 0]).T.astype(np.float32)).astype(
            ml_dtypes.bfloat16)
        for b in range(B)
    ]
    wqTs, wkTs, wvTs, wpTs, bqs = [], [], [], [], []
    for hg in range(2):
        rows = slice(hg * CO, hg * CO + CO)
        wqTs.append(np.ascontiguousarray((Wq[rows] * scale).T).astype(bf16))
        wkTs.append(np.ascontiguousarray(Wkv[rows].T).astype(bf16))
        wvTs.append(np.ascontiguousarray(Wkv[C + hg * CO: C + hg * CO + CO].T).astype(bf16))
        wpTs.append(np.ascontiguousarray(Wp[:, rows].T).astype(bf16))
        bqs.append(np.ascontiguousarray(bq[rows] * scale))

    in_maps = []
    for c in range(N_CORES):
        b, hg = divmod(c, 2)
        in_maps.append({
            "xT": xTs[b], "yT": yTs[b], "m01T": m01Ts[b],
            "wqT": wqTs[hg], "wkT": wkTs[hg], "wvT": wvTs[hg],
            "wpT": wpTs[hg], "bq": bqs[hg],
        })

    nc = _get_nc()
    trace = os.environ.get("CC_ATTN_TRACE", "") == "1"
    res = run_bass_kernel_spmd(nc, in_maps, core_ids=list(range(N_CORES)),
                               trace=trace)
    _CACHE["last_result"] = res

    # host gather: sum the two head-group partials per batch + exact bias folds
    bias = bkv[C:] @ Wp.T + bp  # v-bias passes through softmax exactly
    out = np.empty((B, NQ, C), dtype=np.float32)
    for b in range(B):
        out[b] = (res.results[2 * b]["part"].astype(np.float32)
                  + res.results[2 * b + 1]["part"].astype(np.float32) + bias)
    return out
